# revision 27
# baseline (speedup 1.0000x reference)
"""MoE MLP (top-2 of 8 experts, SwiGLU) on 8 Trainium2 NeuronCores.

Strategy (expert parallelism, per the sharding hint):
  - Host computes router logits once to decide the dispatch (sharding
    decision only), gathers each expert's tokens, and pads to a common
    capacity C.
  - Core e holds expert e's weights (bf16) and runs the 3 matmuls +
    SwiGLU over its gathered tokens, scaling by the combine weight
    on-device.  It also computes the router-logits output for its
    1/8 slice of tokens in fp32 on the tensor engine.
  - Host scatter-adds the per-expert outputs back to token order.

Device layouts (per core; p = 128 SBUF partitions):
  xt   [D, C]   bf16  gathered tokens, transposed (DMAed as [p, 8, C])
  wg   [D, F]   bf16  gate_proj   wu [D, F] bf16 up_proj  ([p, 8, 512]x4)
  wd   [F, D]   bf16  down_proj  (DMAed as [p, 16, D])
  cwt  [p, CT]  f32   combine weights, cwt[p, i] = w[i*128 + p]
  xs   [D, TS]  f32   this core's token slice, transposed ([p, 8, TS])
  gwtp [D, 32]  f32   gate_w.T zero-padded from 8 to 32 experts
outputs:
  yt  [C, D]  f32   weight * expert(token) for the gathered tokens
  lg  [TS, E] f32   router logits slice
"""

import numpy as np
import ml_dtypes

import concourse.mybir as mybir
import concourse.tile as tile
from concourse import bacc
from concourse.bass_utils import run_bass_kernel_spmd

B, S, D, F, E = 2, 2048, 1024, 2048, 8
T = B * S
TS = T // E          # tokens per core for the router-logits output
N_CORES = 8
EPS_TIEBREAK = 1e-6
P = 128              # partitions
NBLK = 512           # matmul moving-dim block (one PSUM bank of fp32)
EP = 32              # router logit rows padded to one DVE transpose block
WARMUP_MM = 16       # PE warmup matmuls to bridge the input-DMA head

BF16 = mybir.dt.bfloat16
F32 = mybir.dt.float32

TRACE = False        # test.py flips this to capture an NTFF profile
LAST_RESULTS = None  # test.py reads exec_time_ns from here

_compiled = {}       # C -> nc


def _blocks(total, blk):
    out = []
    o = 0
    while o < total:
        b = min(blk, total - o)
        out.append((o, b))
        o += b
    return out


def build_bass(C):
    KD = D // P    # 8  contraction chunks over D
    KF = F // P    # 16 contraction chunks over F
    WBLK = F // 4  # Wg/Wu column-block per DMA (512)
    cblocks = _blocks(C, NBLK)
    ctiles = _blocks(C, P)
    CT = len(ctiles)

    nc = bacc.Bacc("TRN2", target_bir_lowering=False, debug=False,
                   num_devices=N_CORES)

    xt = nc.dram_tensor("xt", [D, C], BF16, kind="ExternalInput").ap()
    wg = nc.dram_tensor("wg", [D, F], BF16, kind="ExternalInput").ap()
    wu = nc.dram_tensor("wu", [D, F], BF16, kind="ExternalInput").ap()
    wd = nc.dram_tensor("wd", [F, D], BF16, kind="ExternalInput").ap()
    cwt = nc.dram_tensor("cwt", [P, CT], F32, kind="ExternalInput").ap()
    xs = nc.dram_tensor("xs", [D, TS], F32, kind="ExternalInput").ap()
    gwtp = nc.dram_tensor("gwtp", [D, EP], F32, kind="ExternalInput").ap()

    yt = nc.dram_tensor("yt", [C, D], F32, kind="ExternalOutput").ap()
    lg = nc.dram_tensor("lg", [TS, E], F32, kind="ExternalOutput").ap()

    with tile.TileContext(nc) as tc:
        with (
            tc.tile_pool(name="persist", bufs=1) as pp,
            tc.tile_pool(name="work", bufs=3) as wp,
            tc.tile_pool(name="psum1", bufs=3, space="PSUM") as pq1,
            tc.tile_pool(name="psum2", bufs=2, space="PSUM") as pq,
            tc.tile_pool(name="psum_lg", bufs=1, space="PSUM") as plg,
        ):
            # ---- input DMAs --------------------------------------------
            rings = [nc.sync, nc.scalar, nc.gpsimd]

            def part_load(name, dram_ap, parts, width, dtype, ring_ids):
                """Load [p, a_tot, width] in `parts` (list of a-counts),
                part i on ring ring_ids[i]; returns a -> (tile, local)."""
                look = {}
                a0 = 0
                for ri, an in zip(ring_ids, parts):
                    t = pp.tile([P, an, width], dtype, tag=f"{name}{a0}",
                                name=f"{name}{a0}_sb")
                    rings[ri].dma_start(out=t, in_=dram_ap[:, a0:a0 + an, :])
                    for j in range(an):
                        look[a0 + j] = (t, j)
                    a0 += an
                return look

            # All input DMAs go through one logical HW queue at ~370 GB/s
            # aggregate, draining in issue order — so issue everything on
            # ONE ring in strict consumption order: xs (router) first,
            # then xt, then Wg/Wu column-block pairs in fk order, then
            # tail data (cwt, wd).  Descriptor runs all >= 1 KB.
            xs_l = part_load("xs", xs.rearrange("(a p) c -> p a c", p=P),
                             [KD], TS, F32, [0])
            gwtp_sb = pp.tile([P, KD, EP], F32, tag="gwtp", name="gwtp_sb")
            rings[0].dma_start(out=gwtp_sb,
                               in_=gwtp.rearrange("(a p) e -> p a e", p=P))
            xt_l = part_load("xt", xt.rearrange("(a p) c -> p a c", p=P),
                             [4, 4], C, BF16, [0, 0])
            wgv = wg.rearrange("(a p) f -> p a f", p=P)
            wuv = wu.rearrange("(a p) f -> p a f", p=P)
            wg_l = []
            wu_l = []
            for s in range(4):
                parts = [4, 4] if s == 0 else [KD]
                rids = [0] * len(parts)
                wg_l.append(part_load(f"wgs{s}",
                                      wgv[:, :, s * WBLK:(s + 1) * WBLK],
                                      parts, WBLK, BF16, rids))
                wu_l.append(part_load(f"wus{s}",
                                      wuv[:, :, s * WBLK:(s + 1) * WBLK],
                                      parts, WBLK, BF16, rids))
            cwt_sb = pp.tile([P, CT], F32, tag="cwt", name="cwt_sb")
            rings[0].dma_start(out=cwt_sb, in_=cwt)
            wd_l = part_load("wd", wd.rearrange("(a p) d -> p a d", p=P),
                             [KF], D, BF16, [0])

            # ---- PE warmup: keep HAM busy while inputs stream in -------
            wz = pp.tile([P, NBLK], BF16, tag="wz", name="wz")
            nc.vector.memset(wz, 0.0)
            for i in range(WARMUP_MM):
                pw = pq1.tile([P, NBLK], F32, tag="p1", name=f"pw{i}")
                nc.tensor.matmul(pw, lhsT=wz[:, :P], rhs=wz,
                                 start=True, stop=True)

            # ---- phase A: router logits fp32, transposed [EP, TS] ------
            plT = plg.tile([EP, TS], F32, tag="pl", name="plT")
            for dk in range(KD):
                xs_t, xs_j = xs_l[dk]
                nc.tensor.matmul(plT,
                                 lhsT=gwtp_sb[:, dk, :],
                                 rhs=xs_t[:, xs_j, :],
                                 start=(dk == 0), stop=(dk == KD - 1))
            lgT = wp.tile([EP, TS], F32, tag="lgT", name="lgT")
            nc.scalar.activation(lgT, plT, mybir.ActivationFunctionType.Copy)
            lgB = wp.tile([EP, TS], F32, tag="lgB", name="lgB")
            nc.vector.transpose(lgB, lgT)  # 32x32 blocks: [e, t] -> [t, e]
            nc.sync.dma_start(
                out=lg.rearrange("(a p) e -> p a e", p=EP),
                in_=lgB.rearrange("p (a c) -> p a c", c=EP)[:, :, :E])

            # ---- phase B: hT = silu(Wg.T x) * (Wu.T x), [F, C] bf16 ----
            h_sb = [pp.tile([P, C], BF16, tag=f"h{fk}", name=f"h{fk}")
                    for fk in range(KF)]
            for fk in range(KF):
                s, j = fk // 4, (fk % 4) * P
                for (c0, cb) in cblocks:
                    p1 = pq1.tile([P, NBLK], F32, tag="p1",
                                 name=f"p1_{fk}_{c0}")
                    for dk in range(KD):
                        xt_t, xt_j = xt_l[dk]
                        wg_t, wg_j = wg_l[s][dk]
                        nc.tensor.matmul(
                            p1[:, :cb],
                            lhsT=wg_t[:, wg_j, j:j + P],
                            rhs=xt_t[:, xt_j, c0:c0 + cb],
                            start=(dk == 0), stop=(dk == KD - 1),
                        )
                    s1 = wp.tile([P, NBLK], BF16, tag="s1",
                                 name=f"s1_{fk}_{c0}")
                    nc.scalar.activation(s1[:, :cb], p1[:, :cb],
                                         mybir.ActivationFunctionType.Silu)
                    p3 = pq.tile([P, NBLK], F32, tag="p3",
                                 name=f"p3_{fk}_{c0}")
                    for dk in range(KD):
                        xt_t, xt_j = xt_l[dk]
                        wu_t, wu_j = wu_l[s][dk]
                        nc.tensor.matmul(
                            p3[:, :cb],
                            lhsT=wu_t[:, wu_j, j:j + P],
                            rhs=xt_t[:, xt_j, c0:c0 + cb],
                            start=(dk == 0), stop=(dk == KD - 1),
                        )
                    nc.vector.tensor_mul(h_sb[fk][:, c0:c0 + cb],
                                         s1[:, :cb], p3[:, :cb])

            # ---- phase C: yt = cw * (hT.T @ Wd), [C, D] f32 ------------
            for ci, (t0, tb) in enumerate(ctiles):
                for (d0, db) in _blocks(D, NBLK):
                    po = pq.tile([P, NBLK], F32, tag="po",
                                 name=f"po_{ci}_{d0}")
                    for fk in range(KF):
                        wd_t, wd_j = wd_l[fk]
                        nc.tensor.matmul(
                            po[:tb, :db],
                            lhsT=h_sb[fk][:, t0:t0 + tb],
                            rhs=wd_t[:, wd_j, d0:d0 + db],
                            start=(fk == 0), stop=(fk == KF - 1),
                        )
                    yts = wp.tile([P, NBLK], F32, tag="yts",
                                  name=f"yts_{ci}_{d0}")
                    nc.scalar.activation(yts[:tb, :db], po[:tb, :db],
                                         mybir.ActivationFunctionType.Copy,
                                         scale=cwt_sb[:tb, ci:ci + 1])
                    nc.sync.dma_start(
                        out=yt[t0:t0 + tb, d0:d0 + db],
                        in_=yts[:tb, :db])

    nc.compile()
    return nc


def _get_compiled(C):
    if C not in _compiled:
        _compiled[C] = build_bass(C)
    return _compiled[C]


def kernel(hidden_states, gate_w, Wg, Wu, Wd, top_k=2, step_num=0, **_):
    global LAST_RESULTS
    assert int(top_k) == 2
    x = np.asarray(hidden_states, dtype=np.float32).reshape(T, D)
    gate_w = np.asarray(gate_w, dtype=np.float32)
    Wg = np.asarray(Wg, dtype=np.float32)
    Wu = np.asarray(Wu, dtype=np.float32)
    Wd = np.asarray(Wd, dtype=np.float32)

    # ---- host routing (sharding decision) ----------------------------
    logits = x @ gate_w.T                                    # [T, E]
    comp = -logits + np.arange(E, dtype=np.float32) * EPS_TIEBREAK
    sel = np.argsort(comp, axis=-1, kind="stable")[:, :2]    # [T, 2]
    sl = np.take_along_axis(logits, sel, axis=-1)
    m = sl.max(axis=-1, keepdims=True)
    ew = np.exp(sl - m)
    rw = (ew / ew.sum(axis=-1, keepdims=True)).astype(np.float32)

    idx = [None] * E
    wts = [None] * E
    for e in range(E):
        rows, cols = np.nonzero(sel == e)
        idx[e] = rows
        wts[e] = rw[rows, cols]
    counts = np.array([len(i) for i in idx])
    C = max(P, int(-(-counts.max() // 8)) * 8)     # capacity, multiple of 8
    CT = (C + P - 1) // P

    # ---- per-core inputs ---------------------------------------------
    bf = ml_dtypes.bfloat16
    gwtp = np.zeros((D, EP), dtype=np.float32)
    gwtp[:, :E] = gate_w.T
    in_maps = []
    for e in range(E):
        n = counts[e]
        xtb = np.zeros((D, C), dtype=bf)
        xtb[:, :n] = x[idx[e]].T.astype(bf)
        cwv = np.zeros(CT * P, dtype=np.float32)
        cwv[:n] = wts[e]
        in_maps.append({
            "xt": xtb,
            "wg": Wg[e].astype(bf),
            "wu": Wu[e].astype(bf),
            "wd": Wd[e].astype(bf),
            "cwt": np.ascontiguousarray(cwv.reshape(CT, P).T),
            "xs": np.ascontiguousarray(x[e * TS:(e + 1) * TS].T),
            "gwtp": gwtp,
        })

    nc = _get_compiled(C)
    res = run_bass_kernel_spmd(nc, in_maps, core_ids=list(range(N_CORES)),
                               trace=TRACE)
    LAST_RESULTS = res

    # ---- combine ------------------------------------------------------
    out = np.zeros((T, D), dtype=np.float32)
    for e in range(E):
        n = counts[e]
        out[idx[e]] += res.results[e]["yt"][:n]
    router_logits = np.concatenate(
        [res.results[e]["lg"] for e in range(E)], axis=0)
    return out.reshape(B, S, D), router_logits


# revision 37
# speedup vs baseline: 1.0493x; 1.0493x over previous
"""MoE MLP (top-2 of 8 experts, SwiGLU) on 8 Trainium2 NeuronCores.

Strategy (expert parallelism, per the sharding hint):
  - Host computes router logits once to decide the dispatch (sharding
    decision only), gathers each expert's tokens, and pads to a common
    capacity C.
  - Core e holds expert e's weights (bf16) and runs the 3 matmuls +
    SwiGLU over its gathered tokens, scaling by the combine weight
    on-device.  It also computes the router-logits output for its
    1/8 slice of tokens in fp32 on the tensor engine.
  - Host scatter-adds the per-expert outputs back to token order.

Device layouts (per core; p = 128 SBUF partitions):
  xt   [D, C]   bf16  gathered tokens, transposed (DMAed as [p, 8, C])
  wg   [D, F]   bf16  gate_proj   wu [D, F] bf16 up_proj  ([p, 8, 512]x4)
  wd   [F, D]   bf16  down_proj  (DMAed as [p, 16, D])
  cwb  [p, C]   f32   combine weight per token, replicated on partitions
  xs   [D, TS]  f32   this core's token slice, transposed ([p, 8, TS])
  gwtp [D, 32]  f32   gate_w.T zero-padded from 8 to 32 experts
outputs:
  yt  [D, C]  f32   weight * expert(token), transposed (token-moving)
  lg  [TS, E] f32   router logits slice
"""

import numpy as np
import ml_dtypes

import concourse.mybir as mybir
import concourse.tile as tile
from concourse import bacc
from concourse.bass_utils import run_bass_kernel_spmd

B, S, D, F, E = 2, 2048, 1024, 2048, 8
T = B * S
TS = T // E          # tokens per core for the router-logits output
N_CORES = 8
EPS_TIEBREAK = 1e-6
P = 128              # partitions
NBLK = 512           # matmul moving-dim block (one PSUM bank of fp32)
EP = 32              # router logit rows padded to one DVE transpose block
WARMUP_MM = 16       # PE warmup matmuls to bridge the input-DMA head

BF16 = mybir.dt.bfloat16
F32 = mybir.dt.float32

TRACE = False        # test.py flips this to capture an NTFF profile
LAST_RESULTS = None  # test.py reads exec_time_ns from here

_compiled = {}       # C -> nc


def _blocks(total, blk):
    out = []
    o = 0
    while o < total:
        b = min(blk, total - o)
        out.append((o, b))
        o += b
    return out


def build_bass(C):
    KD = D // P    # 8  contraction chunks over D
    KF = F // P    # 16 contraction chunks over F
    WBLK = F // 4  # Wg/Wu column-block per DMA (512)
    cblocks = _blocks(C, NBLK)

    nc = bacc.Bacc("TRN2", target_bir_lowering=False, debug=False,
                   num_devices=N_CORES)

    xt = nc.dram_tensor("xt", [D, C], BF16, kind="ExternalInput").ap()
    wg = nc.dram_tensor("wg", [D, F], BF16, kind="ExternalInput").ap()
    wu = nc.dram_tensor("wu", [D, F], BF16, kind="ExternalInput").ap()
    wd = nc.dram_tensor("wd", [F, D], BF16, kind="ExternalInput").ap()
    cwb = nc.dram_tensor("cwb", [P, C], F32, kind="ExternalInput").ap()
    xs = nc.dram_tensor("xs", [D, TS], F32, kind="ExternalInput").ap()
    gwtp = nc.dram_tensor("gwtp", [D, EP], F32, kind="ExternalInput").ap()

    yt = nc.dram_tensor("yt", [D, C], F32, kind="ExternalOutput").ap()
    lg = nc.dram_tensor("lg", [TS, E], F32, kind="ExternalOutput").ap()

    with tile.TileContext(nc) as tc:
        with (
            tc.tile_pool(name="persist", bufs=1) as pp,
            tc.tile_pool(name="work", bufs=3) as wp,
            tc.tile_pool(name="psum1", bufs=3, space="PSUM") as pq1,
            tc.tile_pool(name="psum3", bufs=3, space="PSUM") as pq3,
            tc.tile_pool(name="psum2", bufs=2, space="PSUM") as pq,
        ):
            # ---- input DMAs --------------------------------------------
            rings = [nc.sync, nc.scalar, nc.gpsimd]

            def part_load(name, dram_ap, parts, width, dtype, ring_ids):
                """Load [p, a_tot, width] in `parts` (list of a-counts),
                part i on ring ring_ids[i]; returns a -> (tile, local)."""
                look = {}
                a0 = 0
                for ri, an in zip(ring_ids, parts):
                    t = pp.tile([P, an, width], dtype, tag=f"{name}{a0}",
                                name=f"{name}{a0}_sb")
                    rings[ri].dma_start(out=t, in_=dram_ap[:, a0:a0 + an, :])
                    for j in range(an):
                        look[a0 + j] = (t, j)
                    a0 += an
                return look

            # All input DMAs go through one logical HW queue at ~370 GB/s
            # aggregate, draining in issue order — so issue everything on
            # ONE ring in strict consumption order: xs (router) first,
            # then xt, then Wg/Wu column-block pairs in fk order, then
            # tail data (cwt, wd).  Descriptor runs all >= 1 KB.
            xs_l = part_load("xs", xs.rearrange("(a p) c -> p a c", p=P),
                             [KD], TS, F32, [0])
            gwtp_sb = pp.tile([P, KD, EP], F32, tag="gwtp", name="gwtp_sb")
            rings[0].dma_start(out=gwtp_sb,
                               in_=gwtp.rearrange("(a p) e -> p a e", p=P))
            xt_l = part_load("xt", xt.rearrange("(a p) c -> p a c", p=P),
                             [4, 4], C, BF16, [0, 0])
            wgv = wg.rearrange("(a p) f -> p a f", p=P)
            wuv = wu.rearrange("(a p) f -> p a f", p=P)
            wg_l = []
            wu_l = []
            for s in range(4):
                parts = [4, 4] if s == 0 else [KD]
                rids = [0] * len(parts)
                wg_l.append(part_load(f"wgs{s}",
                                      wgv[:, :, s * WBLK:(s + 1) * WBLK],
                                      parts, WBLK, BF16, rids))
                wu_l.append(part_load(f"wus{s}",
                                      wuv[:, :, s * WBLK:(s + 1) * WBLK],
                                      parts, WBLK, BF16, rids))
            cwb_sb = pp.tile([P, C], F32, tag="cwb", name="cwb_sb")
            rings[0].dma_start(out=cwb_sb, in_=cwb)
            wd_l = part_load("wd", wd.rearrange("(a p) d -> p a d", p=P),
                             [KF], D, BF16, [0])

            # ---- PE warmup: keep HAM busy while inputs stream in -------
            wz = pp.tile([P, NBLK], BF16, tag="wz", name="wz")
            nc.vector.memset(wz, 0.0)
            for i in range(WARMUP_MM):
                pw = pq1.tile([P, NBLK], F32, tag="p1", name=f"pw{i}")
                nc.tensor.matmul(pw, lhsT=wz[:, :P], rhs=wz,
                                 start=True, stop=True)

            # ---- phase A: router logits fp32, transposed [EP, TS] ------
            plT = pq.tile([EP, TS], F32, tag="po", name="plT")
            for dk in range(KD):
                xs_t, xs_j = xs_l[dk]
                nc.tensor.matmul(plT,
                                 lhsT=gwtp_sb[:, dk, :],
                                 rhs=xs_t[:, xs_j, :],
                                 start=(dk == 0), stop=(dk == KD - 1))
            lgT = wp.tile([EP, TS], F32, tag="lgT", name="lgT")
            nc.scalar.activation(lgT, plT, mybir.ActivationFunctionType.Copy)
            lgB = wp.tile([EP, TS], F32, tag="lgB", name="lgB")
            nc.vector.transpose(lgB, lgT)  # 32x32 blocks: [e, t] -> [t, e]
            nc.sync.dma_start(
                out=lg.rearrange("(a p) e -> p a e", p=EP),
                in_=lgB.rearrange("p (a c) -> p a c", c=EP)[:, :, :E])

            # ---- phase B: hT = silu(Wg.T x) * (Wu.T x), [F, C] bf16 ----
            h_sb = [pp.tile([P, C], BF16, tag=f"h{fk}", name=f"h{fk}")
                    for fk in range(KF)]
            for fk in range(KF):
                s, j = fk // 4, (fk % 4) * P
                for (c0, cb) in cblocks:
                    p1 = pq1.tile([P, NBLK], F32, tag="p1",
                                 name=f"p1_{fk}_{c0}")
                    for dk in range(KD):
                        xt_t, xt_j = xt_l[dk]
                        wg_t, wg_j = wg_l[s][dk]
                        nc.tensor.matmul(
                            p1[:, :cb],
                            lhsT=wg_t[:, wg_j, j:j + P],
                            rhs=xt_t[:, xt_j, c0:c0 + cb],
                            start=(dk == 0), stop=(dk == KD - 1),
                        )
                    s1 = wp.tile([P, NBLK], BF16, tag="s1",
                                 name=f"s1_{fk}_{c0}")
                    nc.scalar.activation(s1[:, :cb], p1[:, :cb],
                                         mybir.ActivationFunctionType.Silu)
                    p3 = pq3.tile([P, NBLK], F32, tag="p3",
                                 name=f"p3_{fk}_{c0}")
                    for dk in range(KD):
                        xt_t, xt_j = xt_l[dk]
                        wu_t, wu_j = wu_l[s][dk]
                        nc.tensor.matmul(
                            p3[:, :cb],
                            lhsT=wu_t[:, wu_j, j:j + P],
                            rhs=xt_t[:, xt_j, c0:c0 + cb],
                            start=(dk == 0), stop=(dk == KD - 1),
                        )
                    nc.vector.tensor_mul(h_sb[fk][:, c0:c0 + cb],
                                         s1[:, :cb], p3[:, :cb])

            # ---- phase C: ytT = cw ⊙ (Wd.T @ hT), [D, C] f32 -----------
            # Tokens on the moving side: 128*C cycles total, no partial-
            # M-tile waste; combine weight applied in the DVE eviction.
            for dc in range(KD):
                for (c0, cb) in cblocks:
                    po = pq.tile([P, NBLK], F32, tag="po",
                                 name=f"po_{dc}_{c0}")
                    for fk in range(KF):
                        wd_t, wd_j = wd_l[fk]
                        nc.tensor.matmul(
                            po[:, :cb],
                            lhsT=wd_t[:, wd_j, dc * P:(dc + 1) * P],
                            rhs=h_sb[fk][:, c0:c0 + cb],
                            start=(fk == 0), stop=(fk == KF - 1),
                        )
                    yts = wp.tile([P, NBLK], F32, tag="yts",
                                  name=f"yts_{dc}_{c0}")
                    nc.vector.tensor_mul(yts[:, :cb], po[:, :cb],
                                         cwb_sb[:, c0:c0 + cb])
                    nc.sync.dma_start(
                        out=yt[dc * P:(dc + 1) * P, c0:c0 + cb],
                        in_=yts[:, :cb])

    nc.compile()
    return nc


def _get_compiled(C):
    if C not in _compiled:
        _compiled[C] = build_bass(C)
    return _compiled[C]


def kernel(hidden_states, gate_w, Wg, Wu, Wd, top_k=2, step_num=0, **_):
    global LAST_RESULTS
    assert int(top_k) == 2
    x = np.asarray(hidden_states, dtype=np.float32).reshape(T, D)
    gate_w = np.asarray(gate_w, dtype=np.float32)
    Wg = np.asarray(Wg, dtype=np.float32)
    Wu = np.asarray(Wu, dtype=np.float32)
    Wd = np.asarray(Wd, dtype=np.float32)

    # ---- host routing (sharding decision) ----------------------------
    logits = x @ gate_w.T                                    # [T, E]
    comp = -logits + np.arange(E, dtype=np.float32) * EPS_TIEBREAK
    sel = np.argsort(comp, axis=-1, kind="stable")[:, :2]    # [T, 2]
    sl = np.take_along_axis(logits, sel, axis=-1)
    m = sl.max(axis=-1, keepdims=True)
    ew = np.exp(sl - m)
    rw = (ew / ew.sum(axis=-1, keepdims=True)).astype(np.float32)

    idx = [None] * E
    wts = [None] * E
    for e in range(E):
        rows, cols = np.nonzero(sel == e)
        idx[e] = rows
        wts[e] = rw[rows, cols]
    counts = np.array([len(i) for i in idx])
    C = max(P, int(-(-counts.max() // 8)) * 8)     # capacity, multiple of 8

    # ---- per-core inputs ---------------------------------------------
    bf = ml_dtypes.bfloat16
    gwtp = np.zeros((D, EP), dtype=np.float32)
    gwtp[:, :E] = gate_w.T
    in_maps = []
    for e in range(E):
        n = counts[e]
        xtb = np.zeros((D, C), dtype=bf)
        xtb[:, :n] = x[idx[e]].T.astype(bf)
        cwv = np.zeros(C, dtype=np.float32)
        cwv[:n] = wts[e]
        in_maps.append({
            "xt": xtb,
            "wg": Wg[e].astype(bf),
            "wu": Wu[e].astype(bf),
            "wd": Wd[e].astype(bf),
            "cwb": np.ascontiguousarray(
                np.broadcast_to(cwv, (P, C))),
            "xs": np.ascontiguousarray(x[e * TS:(e + 1) * TS].T),
            "gwtp": gwtp,
        })

    nc = _get_compiled(C)
    res = run_bass_kernel_spmd(nc, in_maps, core_ids=list(range(N_CORES)),
                               trace=TRACE)
    LAST_RESULTS = res

    # ---- combine ------------------------------------------------------
    out = np.zeros((T, D), dtype=np.float32)
    for e in range(E):
        n = counts[e]
        out[idx[e]] += np.ascontiguousarray(res.results[e]["yt"][:, :n].T)
    router_logits = np.concatenate(
        [res.results[e]["lg"] for e in range(E)], axis=0)
    return out.reshape(B, S, D), router_logits


# revision 43
# speedup vs baseline: 1.0659x; 1.0158x over previous
"""MoE MLP (top-2 of 8 experts, SwiGLU) on 8 Trainium2 NeuronCores.

Strategy (expert parallelism, per the sharding hint):
  - Host computes router logits once to decide the dispatch (sharding
    decision only), gathers each expert's tokens, and pads to a common
    capacity C.
  - Core e holds expert e's weights (bf16) and runs the 3 matmuls +
    SwiGLU over its gathered tokens, scaling by the combine weight
    on-device.  It also computes the router-logits output for its
    1/8 slice of tokens in fp32 on the tensor engine.
  - Host scatter-adds the per-expert outputs back to token order.

Device layouts (per core; p = 128 SBUF partitions):
  xt   [D, C]   bf16  gathered tokens, transposed (DMAed as [p, 8, C])
  wg   [D, F]   bf16  gate_proj   wu [D, F] bf16 up_proj  ([p, 8, 512]x4)
  wd   [F, D]   bf16  down_proj  (DMAed as [p, 16, D])
  cwb  [p, C]   f32   combine weight per token, replicated on partitions
  xs   [D, TS]  f32   this core's token slice, transposed ([p, 8, TS])
  gwtp [D, 32]  f32   gate_w.T zero-padded from 8 to 32 experts
outputs:
  yt  [D, C]  f32   weight * expert(token), transposed (token-moving)
  lg  [TS, E] f32   router logits slice
"""

import numpy as np
import ml_dtypes

import concourse.mybir as mybir
import concourse.tile as tile
from concourse import bacc
from concourse.bass_utils import run_bass_kernel_spmd

B, S, D, F, E = 2, 2048, 1024, 2048, 8
T = B * S
TS = T // E          # tokens per core for the router-logits output
N_CORES = 8
EPS_TIEBREAK = 1e-6
P = 128              # partitions
NBLK = 512           # matmul moving-dim block (one PSUM bank of fp32)
EP = 32              # router logit rows padded to one DVE transpose block
WARMUP_MM = 16       # PE warmup matmuls to bridge the input-DMA head

BF16 = mybir.dt.bfloat16
F32 = mybir.dt.float32

TRACE = False        # test.py flips this to capture an NTFF profile
LAST_RESULTS = None  # test.py reads exec_time_ns from here

_compiled = {}       # C -> nc


def _blocks(total, blk):
    out = []
    o = 0
    while o < total:
        b = min(blk, total - o)
        out.append((o, b))
        o += b
    return out


def build_bass(C):
    KD = D // P    # 8  contraction chunks over D
    KF = F // P    # 16 contraction chunks over F
    WBLK = F // 4  # Wg/Wu column-block per DMA (512)
    cblocks = _blocks(C, NBLK)

    nc = bacc.Bacc("TRN2", target_bir_lowering=False, debug=False,
                   num_devices=N_CORES)

    xt = nc.dram_tensor("xt", [D, C], BF16, kind="ExternalInput").ap()
    wg = nc.dram_tensor("wg", [D, F], BF16, kind="ExternalInput").ap()
    wu = nc.dram_tensor("wu", [D, F], BF16, kind="ExternalInput").ap()
    wd = nc.dram_tensor("wd", [F, D], BF16, kind="ExternalInput").ap()
    cwb = nc.dram_tensor("cwb", [P, C], F32, kind="ExternalInput").ap()
    xs = nc.dram_tensor("xs", [D, TS], F32, kind="ExternalInput").ap()
    gwtp = nc.dram_tensor("gwtp", [D, EP], F32, kind="ExternalInput").ap()

    yt = nc.dram_tensor("yt", [D, C], F32, kind="ExternalOutput").ap()
    lg = nc.dram_tensor("lg", [TS, E], F32, kind="ExternalOutput").ap()

    with tile.TileContext(nc) as tc:
        with (
            tc.tile_pool(name="persist", bufs=1) as pp,
            tc.tile_pool(name="work", bufs=3) as wp,
            tc.tile_pool(name="psum1", bufs=3, space="PSUM") as pq1,
            tc.tile_pool(name="psum3", bufs=3, space="PSUM") as pq3,
            tc.tile_pool(name="psum2", bufs=2, space="PSUM") as pq,
        ):
            # ---- input DMAs --------------------------------------------
            rings = [nc.sync, nc.scalar, nc.gpsimd]

            def part_load(name, dram_ap, parts, width, dtype, ring_ids):
                """Load [p, a_tot, width] in `parts` (list of a-counts),
                part i on ring ring_ids[i]; returns a -> (tile, local)."""
                look = {}
                a0 = 0
                for ri, an in zip(ring_ids, parts):
                    t = pp.tile([P, an, width], dtype, tag=f"{name}{a0}",
                                name=f"{name}{a0}_sb")
                    rings[ri].dma_start(out=t, in_=dram_ap[:, a0:a0 + an, :])
                    for j in range(an):
                        look[a0 + j] = (t, j)
                    a0 += an
                return look

            # All input DMAs go through one logical HW queue at ~370 GB/s
            # aggregate, draining in issue order — so issue everything on
            # ONE ring in strict consumption order: xs (router) first,
            # then xt, then Wg/Wu column-block pairs in fk order, then
            # tail data (cwt, wd).  Descriptor runs all >= 1 KB.
            xs_l = part_load("xs", xs.rearrange("(a p) c -> p a c", p=P),
                             [KD], TS, F32, [0])
            gwtp_sb = pp.tile([P, KD, EP], F32, tag="gwtp", name="gwtp_sb")
            rings[0].dma_start(out=gwtp_sb,
                               in_=gwtp.rearrange("(a p) e -> p a e", p=P))
            xt_l = part_load("xt", xt.rearrange("(a p) c -> p a c", p=P),
                             [4, 4], C, BF16, [0, 0])
            wgv = wg.rearrange("(a p) f -> p a f", p=P)
            wuv = wu.rearrange("(a p) f -> p a f", p=P)
            wg_l = []
            wu_l = []
            for s in range(4):
                parts = [4, 4] if s == 0 else [KD]
                rids = [0] * len(parts)
                wg_l.append(part_load(f"wgs{s}",
                                      wgv[:, :, s * WBLK:(s + 1) * WBLK],
                                      parts, WBLK, BF16, rids))
                wu_l.append(part_load(f"wus{s}",
                                      wuv[:, :, s * WBLK:(s + 1) * WBLK],
                                      parts, WBLK, BF16, rids))
            cwb_sb = pp.tile([P, C], F32, tag="cwb", name="cwb_sb")
            rings[0].dma_start(out=cwb_sb, in_=cwb)
            wd_l = part_load("wd", wd.rearrange("(a p) d -> p a d", p=P),
                             [KF], D, BF16, [0])

            # ---- PE warmup: keep HAM busy while inputs stream in -------
            wz = pp.tile([P, NBLK], BF16, tag="wz", name="wz")
            nc.vector.memset(wz, 0.0)
            for i in range(WARMUP_MM):
                pw = pq1.tile([P, NBLK], F32, tag="p1", name=f"pw{i}")
                nc.tensor.matmul(pw, lhsT=wz[:, :P], rhs=wz,
                                 start=True, stop=True)

            # ---- phase A: router logits fp32, transposed [EP, TS] ------
            plT = pq.tile([EP, TS], F32, tag="po", name="plT")
            for dk in range(KD):
                xs_t, xs_j = xs_l[dk]
                nc.tensor.matmul(plT,
                                 lhsT=gwtp_sb[:, dk, :],
                                 rhs=xs_t[:, xs_j, :],
                                 start=(dk == 0), stop=(dk == KD - 1))
            lgT = wp.tile([EP, TS], F32, tag="lgT", name="lgT")
            nc.scalar.activation(lgT, plT, mybir.ActivationFunctionType.Copy)
            lgB = wp.tile([EP, TS], F32, tag="lgB", name="lgB")
            nc.vector.transpose(lgB, lgT)  # 32x32 blocks: [e, t] -> [t, e]
            nc.sync.dma_start(
                out=lg.rearrange("(a p) e -> p a e", p=EP),
                in_=lgB.rearrange("p (a c) -> p a c", c=EP)[:, :, :E])

            # ---- phase B: hT = silu(Wg.T x) * (Wu.T x), [F, C] bf16 ----
            h_sb = [pp.tile([P, C], BF16, tag=f"h{fk}", name=f"h{fk}")
                    for fk in range(KF)]
            for fk in range(KF):
                s, j = fk // 4, (fk % 4) * P
                for (c0, cb) in cblocks:
                    p1 = pq1.tile([P, NBLK], F32, tag="p1",
                                 name=f"p1_{fk}_{c0}")
                    for dk in range(KD):
                        xt_t, xt_j = xt_l[dk]
                        wg_t, wg_j = wg_l[s][dk]
                        nc.tensor.matmul(
                            p1[:, :cb],
                            lhsT=wg_t[:, wg_j, j:j + P],
                            rhs=xt_t[:, xt_j, c0:c0 + cb],
                            start=(dk == 0), stop=(dk == KD - 1),
                        )
                    s1 = wp.tile([P, NBLK], BF16, tag="s1",
                                 name=f"s1_{fk}_{c0}")
                    nc.scalar.activation(s1[:, :cb], p1[:, :cb],
                                         mybir.ActivationFunctionType.Silu)
                    p3 = pq3.tile([P, NBLK], F32, tag="p3",
                                 name=f"p3_{fk}_{c0}")
                    for dk in range(KD):
                        xt_t, xt_j = xt_l[dk]
                        wu_t, wu_j = wu_l[s][dk]
                        nc.tensor.matmul(
                            p3[:, :cb],
                            lhsT=wu_t[:, wu_j, j:j + P],
                            rhs=xt_t[:, xt_j, c0:c0 + cb],
                            start=(dk == 0), stop=(dk == KD - 1),
                        )
                    nc.vector.tensor_mul(h_sb[fk][:, c0:c0 + cb],
                                         s1[:, :cb], p3[:, :cb])

            # ---- phase C: ytT = cw ⊙ (Wd.T @ hT), [D, C] f32 -----------
            # Tokens on the moving side: 128*C cycles total, no partial-
            # M-tile waste; combine weight applied in the DVE eviction.
            for dc in range(KD):
                for ci, (c0, cb) in enumerate(cblocks):
                    po = pq1.tile([P, NBLK], F32, tag="p1",
                                 name=f"po_{dc}_{c0}")
                    for fk in range(KF):
                        wd_t, wd_j = wd_l[fk]
                        nc.tensor.matmul(
                            po[:, :cb],
                            lhsT=wd_t[:, wd_j, dc * P:(dc + 1) * P],
                            rhs=h_sb[fk][:, c0:c0 + cb],
                            start=(fk == 0), stop=(fk == KF - 1),
                        )
                    yts = wp.tile([P, NBLK], F32, tag="yts",
                                  name=f"yts_{dc}_{c0}")
                    nc.vector.tensor_mul(yts[:, :cb], po[:, :cb],
                                         cwb_sb[:, c0:c0 + cb])
                    nc.sync.dma_start(
                        out=yt[dc * P:(dc + 1) * P, c0:c0 + cb],
                        in_=yts[:, :cb])

    nc.compile()
    return nc


def _get_compiled(C):
    if C not in _compiled:
        _compiled[C] = build_bass(C)
    return _compiled[C]


def kernel(hidden_states, gate_w, Wg, Wu, Wd, top_k=2, step_num=0, **_):
    global LAST_RESULTS
    assert int(top_k) == 2
    x = np.asarray(hidden_states, dtype=np.float32).reshape(T, D)
    gate_w = np.asarray(gate_w, dtype=np.float32)
    Wg = np.asarray(Wg, dtype=np.float32)
    Wu = np.asarray(Wu, dtype=np.float32)
    Wd = np.asarray(Wd, dtype=np.float32)

    # ---- host routing (sharding decision) ----------------------------
    logits = x @ gate_w.T                                    # [T, E]
    comp = -logits + np.arange(E, dtype=np.float32) * EPS_TIEBREAK
    sel = np.argsort(comp, axis=-1, kind="stable")[:, :2]    # [T, 2]
    sl = np.take_along_axis(logits, sel, axis=-1)
    m = sl.max(axis=-1, keepdims=True)
    ew = np.exp(sl - m)
    rw = (ew / ew.sum(axis=-1, keepdims=True)).astype(np.float32)

    idx = [None] * E
    wts = [None] * E
    for e in range(E):
        rows, cols = np.nonzero(sel == e)
        idx[e] = rows
        wts[e] = rw[rows, cols]
    counts = np.array([len(i) for i in idx])
    C = max(P, int(-(-counts.max() // 8)) * 8)     # capacity, multiple of 8

    # ---- per-core inputs ---------------------------------------------
    bf = ml_dtypes.bfloat16
    gwtp = np.zeros((D, EP), dtype=np.float32)
    gwtp[:, :E] = gate_w.T
    in_maps = []
    for e in range(E):
        n = counts[e]
        xtb = np.zeros((D, C), dtype=bf)
        xtb[:, :n] = x[idx[e]].T.astype(bf)
        cwv = np.zeros(C, dtype=np.float32)
        cwv[:n] = wts[e]
        in_maps.append({
            "xt": xtb,
            "wg": Wg[e].astype(bf),
            "wu": Wu[e].astype(bf),
            "wd": Wd[e].astype(bf),
            "cwb": np.ascontiguousarray(
                np.broadcast_to(cwv, (P, C))),
            "xs": np.ascontiguousarray(x[e * TS:(e + 1) * TS].T),
            "gwtp": gwtp,
        })

    nc = _get_compiled(C)
    res = run_bass_kernel_spmd(nc, in_maps, core_ids=list(range(N_CORES)),
                               trace=TRACE)
    LAST_RESULTS = res

    # ---- combine ------------------------------------------------------
    out = np.zeros((T, D), dtype=np.float32)
    for e in range(E):
        n = counts[e]
        out[idx[e]] += np.ascontiguousarray(res.results[e]["yt"][:, :n].T)
    router_logits = np.concatenate(
        [res.results[e]["lg"] for e in range(E)], axis=0)
    return out.reshape(B, S, D), router_logits
